# revision 4
# baseline (speedup 1.0000x reference)
"""MoE layer (B=2,T=1024,D=2048,F=768,E=16,K=2) on 8 NeuronCores.

Expert-parallel with load-balanced slots: slot0 = the 8 largest experts
(capacity C0), slot1 = the 8 smallest (capacity C1 <= C0), one of each per
core. Host computes the router (~0.3% of FLOPs), gathers each expert's
tokens into fixed-capacity transposed buffers, and the device kernel runs
the sparse SwiGLU FFN in bf16 with f32 PSUM accumulation.

Schedule (per core, derived from baseline trace analysis):
- slot0's tokens/weights are partition-striped: rows 0-63 of every tile on
  the sync HWDGE ring, rows 64-127 on the scalar ring, enqueued in exact
  demand order (tokens+gate0 first, then gt/ut zippered) so neither ring
  holds not-yet-needed bytes in front of needed ones.
- gate/up chunks consumed in zipper order g0 g1 u0 g2 u1 ... u5 to match
  arrival; silu runs right after each g chunk (frees its PSUM early).
- slot1's loads all ride the sync ring behind slot0's (huge slack), so the
  scalar/ACT engine stays free for silu; slot0 down-weights split gpsimd
  (rows 0-63) + scalar (rows 64-127); slot1 down-weights on sync.
- PE warmup: memset on gpsimd (earliest engine up) + 8 garbage matmuls
  before real work, plus small fillers between early accumulation groups
  so the HAM clock-gate never sees a 3.4us idle window and re-throttles.
- outputs yT [D, C] ride gpsimd SWDGE; the last expert's final two
  m-chunks go out as two small parallel DMAs on sync+scalar to cut the
  post-compute tail.
"""

import numpy as np
from contextlib import ExitStack

import concourse.bass as bass
import concourse.tile as tile
from concourse import mybir
from concourse.bass_utils import run_bass_kernel_spmd

B, T, D, F, E, TOPK = 2, 1024, 2048, 768, 16, 2
NCORES = 8
EPC = E // NCORES  # expert slots per core
P = 128
KD = D // P  # 16 k-tiles over D
KF = F // P  # 6 f-chunks over F
MD = D // P  # 16 m-chunks over D (down proj, yT layout)


def _split_waits(nc, max_waits=1):
    """walrus on this image rejects >1 sync-wait per instruction
    (setupSyncWait: "Too many sync wait commands"); split extras into
    preceding same-engine NoOps."""
    for f in nc.m.functions:
        for b in f.blocks:
            insts = b.instructions
            idx = 0
            while idx < len(insts):
                inst = insts[idx]
                si = getattr(inst, "sync_info", None)
                if si is not None and si.on_wait and len(si.on_wait) > max_waits:
                    waits = list(si.on_wait)
                    extra, keep = waits[:-max_waits], waits[-max_waits:]
                    pos = idx
                    for j in range(0, len(extra), max_waits):
                        chunk = extra[j : j + max_waits]
                        nop = mybir.InstNoOp(name=f"{inst.name}_ws{j}", ins=[], outs=[])
                        nop.engine = inst.engine
                        nop.sync_info = mybir.SyncInfo(on_wait=chunk, on_update=[])
                        insts.insert(pos, nop)
                        pos += 1
                        idx += 1
                    inst.sync_info = mybir.SyncInfo(
                        on_wait=keep, on_update=list(si.on_update)
                    )
                idx += 1


def build_moe(C0, C1):
    """Per-core kernel: slot0 capacity C0, slot1 capacity C1 (both %8==0)."""
    assert C0 % 8 == 0 and C1 % 8 == 0 and C1 <= C0 <= 512
    caps = (C0, C1)
    bf16 = mybir.dt.bfloat16
    f32 = mybir.dt.float32
    H = P // 2  # partition-stripe boundary

    nc = bass.Bass("TRN2", target_bir_lowering=False, debug=False, num_devices=NCORES)
    # host pre-tiled layouts (>=2KB contiguous per partition per DMA):
    #   xg[p, k*C + c] = x_gathered[k*128+p, c]
    #   wg/wu[e, j, p, k*128+f] = w[e, k*128+p, j*128+f]   (slab per f-chunk j)
    xg0 = nc.declare_dram_parameter("xg0", [P, KD * C0], bf16, isOutput=False)
    xg1 = nc.declare_dram_parameter("xg1", [P, KD * C1], bf16, isOutput=False)
    cw0 = nc.declare_dram_parameter("cw0", [P, C0], f32, isOutput=False)
    cw1 = nc.declare_dram_parameter("cw1", [P, C1], f32, isOutput=False)
    wg = nc.declare_dram_parameter("wg", [EPC, KF, P, KD * P], bf16, isOutput=False)
    wu = nc.declare_dram_parameter("wu", [EPC, KF, P, KD * P], bf16, isOutput=False)
    wd = nc.declare_dram_parameter("wd", [EPC, F, D], bf16, isOutput=False)
    y0 = nc.declare_dram_parameter("y0", [D, C0], bf16, isOutput=True)
    y1 = nc.declare_dram_parameter("y1", [D, C1], bf16, isOutput=True)

    with tile.TileContext(nc) as tc, ExitStack() as ctx:
        xp = ctx.enter_context(tc.tile_pool(name="xp", bufs=1))
        wp = ctx.enter_context(tc.tile_pool(name="wp", bufs=1))
        wdp = ctx.enter_context(tc.tile_pool(name="wdp", bufs=1))
        hp = ctx.enter_context(tc.tile_pool(name="hp", bufs=1))
        sp = ctx.enter_context(tc.tile_pool(name="sp", bufs=2))
        cp = ctx.enter_context(tc.tile_pool(name="cp", bufs=1))
        op = ctx.enter_context(tc.tile_pool(name="op", bufs=3))
        pg = ctx.enter_context(tc.tile_pool(name="pg", bufs=3, space="PSUM"))
        pu = ctx.enter_context(tc.tile_pool(name="pu", bufs=2, space="PSUM"))
        py = ctx.enter_context(tc.tile_pool(name="py", bufs=3, space="PSUM"))

        # PE warmup: garbage matmuls with no data deps run during the DMA
        # ramp so HAM un-throttles (1.2->2.4GHz) before real work. memset on
        # gpsimd -- the earliest engine to come up after the preamble.
        wsb = cp.tile([P, 512], bf16, tag="warm_sb")
        nc.gpsimd.memset(wsb[:], 0)
        for _ in range(8):
            wps = py.tile([P, 512], f32, tag="y_ps")
            nc.tensor.matmul(wps[:], wsb[:, :P], wsb[:], start=True, stop=True)

        def filler(n, cols=256):
            for _ in range(n):
                wps = py.tile([P, 512], f32, tag="y_ps")
                nc.tensor.matmul(
                    wps[:, :cols], wsb[:, :P], wsb[:, :cols], start=True, stop=True
                )

        def load2(t, src):
            """Partition-striped load: rows 0:64 via sync ring, 64:128 via
            scalar ring. Consumers (all-partition reads) wait on both."""
            nc.sync.dma_start(t[0:H, :], src[0:H])
            nc.scalar.dma_start(t[H:P, :], src[H:P])

        # ---- slot0 loads: both HWDGE rings, exact demand order ----
        XC = 4  # token column chunks (KD/XC k-tiles each)
        KX = KD // XC
        xts0 = []
        for h in range(XC):
            xt = xp.tile([P, KX * C0], bf16, tag=f"xt0_{h}")
            xts0.append(xt)
        gts0, uts0 = [], []
        for j in range(KF):
            gt = wp.tile([P, KD * P], bf16, tag=f"gt0_{j}")
            gts0.append(gt)
            ut = wp.tile([P, KD * P], bf16, tag=f"ut0_{j}")
            uts0.append(ut)
        # demand order: tokens + gate0 interleaved, then zipper g1 u0 g2 u1 ...
        load2(xts0[0], xg0[:, 0 : KX * C0])
        nc.sync.dma_start(gts0[0][0:H, 0 : 8 * P], wg[0, 0][0:H, 0 : 8 * P])
        nc.scalar.dma_start(gts0[0][H:P, 0 : 8 * P], wg[0, 0][H:P, 0 : 8 * P])
        load2(xts0[1], xg0[:, KX * C0 : 2 * KX * C0])
        nc.sync.dma_start(gts0[0][0:H, 8 * P :], wg[0, 0][0:H, 8 * P :])
        nc.scalar.dma_start(gts0[0][H:P, 8 * P :], wg[0, 0][H:P, 8 * P :])
        load2(xts0[2], xg0[:, 2 * KX * C0 : 3 * KX * C0])
        load2(gts0[1], wg[0, 1])
        load2(xts0[3], xg0[:, 3 * KX * C0 :])
        load2(uts0[0], wu[0, 0])
        for j in range(2, KF):
            load2(gts0[j], wg[0, j])
            load2(uts0[j - 2], wu[0, j - 2])
        load2(uts0[KF - 2], wu[0, KF - 2])
        load2(uts0[KF - 1], wu[0, KF - 1])

        # slot0 down weights: rows 0:64 on gpsimd (free early), 64:128 on
        # scalar behind slot0's R-halves.
        dts0 = []
        wdr0 = wd[0].rearrange("(k p) d -> p k d", p=P)
        for h in range(3):
            dt = wdp.tile([P, KF // 3, D], bf16, tag=f"dt0_{h}")
            nc.gpsimd.dma_start(dt[0:H, :, :], wdr0[0:H, bass.ts(h, KF // 3), :])
            nc.scalar.dma_start(dt[H:P, :, :], wdr0[H:P, bass.ts(h, KF // 3), :])
            dts0.append(dt)

        # routing weights (host-replicated across partitions)
        cwb0 = cp.tile([P, C0], f32, tag="cwb0")
        nc.gpsimd.dma_start(cwb0[:], cw0[:])
        cwb1 = cp.tile([P, C1], f32, tag="cwb1")
        nc.gpsimd.dma_start(cwb1[:], cw1[:])

        # ---- slot1 loads: all on the sync ring behind slot0's L-halves
        # (slot1 compute starts ~35us later; scalar/ACT stays free for silu)
        xt1 = xp.tile([P, KD * C1], bf16, tag="xt1")
        nc.sync.dma_start(xt1[:], xg1[:])
        gts1, uts1 = [], []
        for j in range(KF):
            gt = wp.tile([P, KD * P], bf16, tag=f"gt1_{j}")
            nc.sync.dma_start(gt[:], wg[1, j])
            gts1.append(gt)
            ut = wp.tile([P, KD * P], bf16, tag=f"ut1_{j}")
            nc.sync.dma_start(ut[:], wu[1, j])
            uts1.append(ut)
        dts1 = []
        wdr1 = wd[1].rearrange("(k p) d -> p k d", p=P)
        for h in range(3):
            dt = wdp.tile([P, KF // 3, D], bf16, tag=f"dt1_{h}")
            nc.sync.dma_start(dt[:], wdr1[:, bass.ts(h, KF // 3), :])
            dts1.append(dt)

        ZIP = [("g", 0), ("g", 1), ("u", 0), ("g", 2), ("u", 1), ("g", 3),
               ("u", 2), ("g", 4), ("u", 3), ("g", 5), ("u", 4), ("u", 5)]
        # fillers after the i-th zipper step of slot0 (supply-bound ramp)
        FILL = {0: 2, 1: 2, 2: 2, 3: 1, 4: 1, 5: 1}

        hts = []
        for e in range(EPC):
            C = caps[e]
            xts = xts0 if e == 0 else [xt1]
            kx = KX if e == 0 else KD
            gts = gts0 if e == 0 else gts1
            uts = uts0 if e == 0 else uts1

            # ---- gate/up + SwiGLU -> hT [F, C] bf16, zipper order ----
            ht = hp.tile([P, KF, C], bf16, tag=f"ht{e}")
            hts.append(ht)
            sils = {}
            for step, (kind, j) in enumerate(ZIP):
                if kind == "g":
                    ps = pg.tile([P, C], f32, tag="g_ps")
                    for k in range(KD):
                        nc.tensor.matmul(
                            ps[:],
                            gts[j][:, bass.ts(k, P)],
                            xts[k // kx][:, bass.ts(k % kx, C)],
                            start=(k == 0),
                            stop=(k == KD - 1),
                        )
                    sil = sp.tile([P, C], f32, tag="sil")
                    nc.scalar.activation(
                        sil[:], ps[:], mybir.ActivationFunctionType.Silu
                    )
                    sils[j] = sil
                else:
                    ps = pu.tile([P, C], f32, tag="u_ps")
                    for k in range(KD):
                        nc.tensor.matmul(
                            ps[:],
                            uts[j][:, bass.ts(k, P)],
                            xts[k // kx][:, bass.ts(k % kx, C)],
                            start=(k == 0),
                            stop=(k == KD - 1),
                        )
                    nc.vector.tensor_mul(ht[:, j, :], sils.pop(j)[:], ps[:])
                if e == 0 and step in FILL:
                    filler(FILL[step])

            # ---- down proj: yT[m-chunk, :] = sum_j wd[j,m].T @ hT[j] ----
            dts = dts0 if e == 0 else dts1
            cwb = cwb0 if e == 0 else cwb1
            y = y0 if e == 0 else y1
            ydst = y.rearrange("(m p) c -> p m c", p=P)
            if e < EPC - 1:
                batches = [(0, 4, nc.gpsimd), (4, 4, nc.gpsimd),
                           (8, 4, nc.gpsimd), (12, 4, nc.gpsimd)]
            else:
                # shrink + parallelize the final flush to cut the tail
                batches = [(0, 4, nc.gpsimd), (4, 4, nc.gpsimd),
                           (8, 4, nc.gpsimd), (12, 2, nc.gpsimd),
                           (14, 1, nc.sync), (15, 1, nc.scalar)]
            for m0, nb, yeng in batches:
                ysb = op.tile([P, nb, C], bf16, tag="ysb")
                for mi in range(nb):
                    m = m0 + mi
                    y_ps = py.tile([P, C], f32, tag="y_ps")
                    for j in range(KF):
                        nc.tensor.matmul(
                            y_ps[:],
                            dts[j // (KF // 3)][:, j % (KF // 3), bass.ts(m, P)],
                            ht[:, j, :],
                            start=(j == 0),
                            stop=(j == KF - 1),
                        )
                    nc.vector.tensor_mul(ysb[:, mi, :], y_ps[:], cwb[:])
                yeng.dma_start(ydst[:, m0 : m0 + nb, :], ysb[:])

    _split_waits(nc)
    return nc


_CACHE = {}


def _get_nc(C0, C1):
    if (C0, C1) not in _CACHE:
        _CACHE[(C0, C1)] = build_moe(C0, C1)
    return _CACHE[(C0, C1)]


def _route(x, router_w):
    """Replicates the reference router in f32: softmax over expert scores,
    top-2, renormalize."""
    xf = x.reshape(-1, D).astype(np.float32)
    scores = xf @ router_w.astype(np.float32)
    m = scores.max(axis=-1, keepdims=True)
    ex = np.exp(scores - m)
    probs = ex / ex.sum(axis=-1, keepdims=True)
    idx = np.argsort(-probs, axis=-1, kind="stable")[:, :TOPK]
    wts = np.take_along_axis(probs, idx, axis=-1)
    wts = wts / wts.sum(axis=-1, keepdims=True)
    return idx.astype(np.int32), wts.astype(np.float32)


def _round8(n):
    return max(8, -(-n // 8) * 8)


def kernel(x, router_w, gate_w, up_w, down_w):
    import ml_dtypes

    bf = ml_dtypes.bfloat16

    x = np.asarray(x)
    in_dtype = x.dtype
    xf = x.reshape(-1, D).astype(np.float32)
    idx, wts = _route(x, np.asarray(router_w))

    # token lists per expert
    tok_ids = [None] * E
    tok_wts = [None] * E
    counts = np.zeros(E, dtype=np.int64)
    for e in range(E):
        sel = np.nonzero(idx == e)
        tok_ids[e] = sel[0].astype(np.int64)
        tok_wts[e] = wts[sel[0], sel[1]]
        counts[e] = len(tok_ids[e])

    # load-balanced slots: slot0 = 8 largest experts, slot1 = 8 smallest;
    # core c processes (desc[c], asc[c]).
    order = np.argsort(-counts, kind="stable")
    slot0 = order[:NCORES]
    slot1 = order[NCORES:][::-1]  # ascending counts
    C0 = min(512, _round8(int(counts[slot0].max())))
    C1 = min(512, _round8(int(counts[slot1].max())))

    nc = _get_nc(C0, C1)

    def tile_gateup(w):
        # [E, D, F] -> [E, KF, P, KD*P] with w_t[e,j,p,k*P+f] = w[e,k*P+p,j*P+f]
        w = np.asarray(w).astype(bf)
        w = w.reshape(E, KD, P, KF, P).transpose(0, 3, 2, 1, 4)
        return np.ascontiguousarray(w.reshape(E, KF, P, KD * P))

    g16 = tile_gateup(gate_w)
    u16 = tile_gateup(up_w)
    d16 = np.asarray(down_w).astype(bf)
    xT = np.ascontiguousarray(xf.T)  # [D, B*T] f32

    def gather(e, C):
        xg = np.zeros((P, KD, C), dtype=bf)
        cwv = np.zeros((P, C), dtype=np.float32)
        n = counts[e]
        gath = xT[:, tok_ids[e]]  # [D, n] f32
        xg[:, :, :n] = gath.astype(bf).reshape(KD, P, n).transpose(1, 0, 2)
        cwv[:, :n] = tok_wts[e][None, :]
        return xg.reshape(P, KD * C), cwv

    in_maps = []
    for c in range(NCORES):
        e0, e1 = int(slot0[c]), int(slot1[c])
        xg0, cwv0 = gather(e0, C0)
        xg1, cwv1 = gather(e1, C1)
        pair = [e0, e1]
        in_maps.append(
            {
                "xg0": xg0,
                "xg1": xg1,
                "cw0": cwv0,
                "cw1": cwv1,
                "wg": np.ascontiguousarray(g16[pair]),
                "wu": np.ascontiguousarray(u16[pair]),
                "wd": np.ascontiguousarray(d16[pair]),
            }
        )

    res = run_bass_kernel_spmd(nc, in_maps, list(range(NCORES)))

    out = np.zeros((B * T, D), dtype=np.float32)
    for c in range(NCORES):
        e0, e1 = int(slot0[c]), int(slot1[c])
        for e, name in ((e0, "y0"), (e1, "y1")):
            yv = res.results[c][name]  # [D, C] bf16
            n = counts[e]
            out[tok_ids[e]] += yv[:, :n].astype(np.float32).T
    return out.reshape(B, T, D).astype(in_dtype)


# revision 6
# speedup vs baseline: 1.1055x; 1.1055x over previous
"""MoE layer (B=2,T=1024,D=2048,F=768,E=16,K=2) on 8 NeuronCores.

Expert-parallel with load-balanced slots: slot0 = the 8 largest experts
(capacity C0), slot1 = the 8 smallest (capacity C1 <= C0), one of each per
core. Host computes the router (~0.3% of FLOPs), gathers each expert's
tokens into fixed-capacity transposed buffers, and the device kernel runs
the sparse SwiGLU FFN in bf16 with f32 PSUM accumulation.

Schedule (per core, derived from trace analysis):
- slot0 tokens (4 col-slabs, queued first) + up slabs ride the scalar HWDGE
  ring; gate slabs (gt0 split in two for an early first matmul) + down
  weights + ALL slot1 gate/up/tokens ride the sync ring; cw + slot1 down
  weights + outputs ride gpsimd SWDGE. Only 10 scalar triggers precede the
  silus so the ACT engine is never blocked behind its DMA ring.
- gate/up chunks consumed in zipper order g0 g1 u0 g2 u1 ... u5 to match
  per-ring FIFO arrival; silu runs right after each g chunk.
- PE warmup: memset on gpsimd (earliest engine up) + 8 garbage matmuls,
  plus 256-col filler matmuls at slab boundaries of the first chunks so
  the HAM clock-gate never sees a 3.4us idle window and re-throttles
  (supply dribbles during the DMA ramp; fillers make the stalls cheap).
- balanced slots need only (296+256) matmul columns per core instead of
  2x296: slot0 = the 8 largest experts, slot1 = the 8 smallest.
- the last expert's final two m-chunks go out as two small parallel DMAs
  on sync+scalar to cut the post-compute tail.
"""

import numpy as np
from contextlib import ExitStack

import concourse.bass as bass
import concourse.tile as tile
from concourse import mybir
from concourse.bass_utils import run_bass_kernel_spmd

B, T, D, F, E, TOPK = 2, 1024, 2048, 768, 16, 2
NCORES = 8
EPC = E // NCORES  # expert slots per core
P = 128
KD = D // P  # 16 k-tiles over D
KF = F // P  # 6 f-chunks over F
MD = D // P  # 16 m-chunks over D (down proj, yT layout)


def _split_waits(nc, max_waits=1):
    """walrus on this image rejects >1 sync-wait per instruction
    (setupSyncWait: "Too many sync wait commands"); split extras into
    preceding same-engine NoOps."""
    for f in nc.m.functions:
        for b in f.blocks:
            insts = b.instructions
            idx = 0
            while idx < len(insts):
                inst = insts[idx]
                si = getattr(inst, "sync_info", None)
                if si is not None and si.on_wait and len(si.on_wait) > max_waits:
                    waits = list(si.on_wait)
                    extra, keep = waits[:-max_waits], waits[-max_waits:]
                    pos = idx
                    for j in range(0, len(extra), max_waits):
                        chunk = extra[j : j + max_waits]
                        nop = mybir.InstNoOp(name=f"{inst.name}_ws{j}", ins=[], outs=[])
                        nop.engine = inst.engine
                        nop.sync_info = mybir.SyncInfo(on_wait=chunk, on_update=[])
                        insts.insert(pos, nop)
                        pos += 1
                        idx += 1
                    inst.sync_info = mybir.SyncInfo(
                        on_wait=keep, on_update=list(si.on_update)
                    )
                idx += 1


def build_moe(C0, C1):
    """Per-core kernel: slot0 capacity C0, slot1 capacity C1 (both %8==0)."""
    assert C0 % 8 == 0 and C1 % 8 == 0 and C1 <= C0 <= 512
    caps = (C0, C1)
    bf16 = mybir.dt.bfloat16
    f32 = mybir.dt.float32
    H = P // 2  # partition-stripe boundary

    nc = bass.Bass("TRN2", target_bir_lowering=False, debug=False, num_devices=NCORES)
    # host pre-tiled layouts (>=2KB contiguous per partition per DMA):
    #   xg[p, k*C + c] = x_gathered[k*128+p, c]
    #   wg/wu[e, j, p, k*128+f] = w[e, k*128+p, j*128+f]   (slab per f-chunk j)
    xg0 = nc.declare_dram_parameter("xg0", [P, KD * C0], bf16, isOutput=False)
    xg1 = nc.declare_dram_parameter("xg1", [P, KD * C1], bf16, isOutput=False)
    cw0 = nc.declare_dram_parameter("cw0", [P, C0], f32, isOutput=False)
    cw1 = nc.declare_dram_parameter("cw1", [P, C1], f32, isOutput=False)
    wg = nc.declare_dram_parameter("wg", [EPC, KF, P, KD * P], bf16, isOutput=False)
    wu = nc.declare_dram_parameter("wu", [EPC, KF, P, KD * P], bf16, isOutput=False)
    wd = nc.declare_dram_parameter("wd", [EPC, F, D], bf16, isOutput=False)
    y0 = nc.declare_dram_parameter("y0", [D, C0], bf16, isOutput=True)
    y1 = nc.declare_dram_parameter("y1", [D, C1], bf16, isOutput=True)

    with tile.TileContext(nc) as tc, ExitStack() as ctx:
        xp = ctx.enter_context(tc.tile_pool(name="xp", bufs=1))
        wp = ctx.enter_context(tc.tile_pool(name="wp", bufs=1))
        wdp = ctx.enter_context(tc.tile_pool(name="wdp", bufs=1))
        hp = ctx.enter_context(tc.tile_pool(name="hp", bufs=1))
        sp = ctx.enter_context(tc.tile_pool(name="sp", bufs=2))
        cp = ctx.enter_context(tc.tile_pool(name="cp", bufs=1))
        op = ctx.enter_context(tc.tile_pool(name="op", bufs=3))
        pg = ctx.enter_context(tc.tile_pool(name="pg", bufs=3, space="PSUM"))
        pu = ctx.enter_context(tc.tile_pool(name="pu", bufs=2, space="PSUM"))
        py = ctx.enter_context(tc.tile_pool(name="py", bufs=3, space="PSUM"))

        # PE warmup: garbage matmuls with no data deps run during the DMA
        # ramp so HAM un-throttles (1.2->2.4GHz) before real work. memset on
        # gpsimd -- the earliest engine to come up after the preamble.
        wsb = cp.tile([P, 512], bf16, tag="warm_sb")
        nc.gpsimd.memset(wsb[:], 0)
        for _ in range(8):
            wps = py.tile([P, 512], f32, tag="y_ps")
            nc.tensor.matmul(wps[:], wsb[:, :P], wsb[:], start=True, stop=True)

        def filler(n, cols=256):
            for _ in range(n):
                wps = py.tile([P, 512], f32, tag="y_ps")
                nc.tensor.matmul(
                    wps[:, :cols], wsb[:, :P], wsb[:, :cols], start=True, stop=True
                )

        # ---- slot0 loads ----
        # tokens: 4 column-slabs on the scalar HWDGE ring, first in queue so
        # the gate k-loop can dribble-start during the DMA ramp.
        XC = 4  # token column chunks (KD/XC k-tiles each)
        KX = KD // XC
        xts0 = []
        for h in range(XC):
            xt = xp.tile([P, KX * C0], bf16, tag=f"xt0_{h}")
            nc.scalar.dma_start(xt[:], xg0[:, bass.ts(h, KX * C0)])
            xts0.append(xt)
        # gate slabs on the sync ring; gt0 split in two so the first real
        # matmul starts ~1us earlier.
        gts0, uts0 = [], []
        for j in range(KF):
            gt = wp.tile([P, KD * P], bf16, tag=f"gt0_{j}")
            if j == 0:
                nc.sync.dma_start(gt[:, 0 : 8 * P], wg[0, 0][:, 0 : 8 * P])
                nc.sync.dma_start(gt[:, 8 * P :], wg[0, 0][:, 8 * P :])
            else:
                nc.sync.dma_start(gt[:], wg[0, j])
            gts0.append(gt)
        # up slabs on the scalar ring (idle after tokens; no silu yet) --
        # exactly 10 scalar triggers before the first silu, below the HWDGE
        # ring depth, so the ACT engine is never blocked behind its ring.
        for j in range(KF):
            ut = wp.tile([P, KD * P], bf16, tag=f"ut0_{j}")
            nc.scalar.dma_start(ut[:], wu[0, j])
            uts0.append(ut)
        # slot0 down weights behind the gate slabs on sync
        dts0 = []
        wdr0 = wd[0].rearrange("(k p) d -> p k d", p=P)
        for h in range(3):
            dt = wdp.tile([P, KF // 3, D], bf16, tag=f"dt0_{h}")
            nc.sync.dma_start(dt[:], wdr0[:, bass.ts(h, KF // 3), :])
            dts0.append(dt)

        # routing weights (host-replicated across partitions) + slot1 down
        # weights on gpsimd SWDGE (idle early, done long before needed)
        cwb0 = cp.tile([P, C0], f32, tag="cwb0")
        nc.gpsimd.dma_start(cwb0[:], cw0[:])
        cwb1 = cp.tile([P, C1], f32, tag="cwb1")
        nc.gpsimd.dma_start(cwb1[:], cw1[:])
        dts1 = []
        wdr1 = wd[1].rearrange("(k p) d -> p k d", p=P)
        for h in range(3):
            dt = wdp.tile([P, KF // 3, D], bf16, tag=f"dt1_{h}")
            nc.gpsimd.dma_start(dt[:], wdr1[:, bass.ts(h, KF // 3), :])
            dts1.append(dt)

        # ---- slot1 gate/up/tokens: all on the sync ring behind slot0's
        # (slot1 compute starts ~35us in; keeps ACT free for silu)
        xt1 = xp.tile([P, KD * C1], bf16, tag="xt1")
        nc.sync.dma_start(xt1[:], xg1[:])
        gts1, uts1 = [], []
        for j in range(KF):
            gt = wp.tile([P, KD * P], bf16, tag=f"gt1_{j}")
            nc.sync.dma_start(gt[:], wg[1, j])
            gts1.append(gt)
            ut = wp.tile([P, KD * P], bf16, tag=f"ut1_{j}")
            nc.sync.dma_start(ut[:], wu[1, j])
            uts1.append(ut)

        ZIP = [("g", 0), ("g", 1), ("u", 0), ("g", 2), ("u", 1), ("g", 3),
               ("u", 2), ("g", 4), ("u", 3), ("g", 5), ("u", 4), ("u", 5)]
        # fillers after the i-th zipper step of slot0 (supply-bound ramp)
        FILL = {0: 2, 1: 2, 2: 2, 3: 1, 4: 1, 5: 1}

        hts = []
        for e in range(EPC):
            C = caps[e]
            xts = xts0 if e == 0 else [xt1]
            kx = KX if e == 0 else KD
            gts = gts0 if e == 0 else gts1
            uts = uts0 if e == 0 else uts1

            # ---- gate/up + SwiGLU -> hT [F, C] bf16, zipper order ----
            ht = hp.tile([P, KF, C], bf16, tag=f"ht{e}")
            hts.append(ht)
            sils = {}
            for step, (kind, j) in enumerate(ZIP):
                if kind == "g":
                    ps = pg.tile([P, C], f32, tag="g_ps")
                    for k in range(KD):
                        if e == 0 and step < 3 and k and k % KX == 0:
                            filler(1)
                        nc.tensor.matmul(
                            ps[:],
                            gts[j][:, bass.ts(k, P)],
                            xts[k // kx][:, bass.ts(k % kx, C)],
                            start=(k == 0),
                            stop=(k == KD - 1),
                        )
                    sil = sp.tile([P, C], f32, tag="sil")
                    nc.scalar.activation(
                        sil[:], ps[:], mybir.ActivationFunctionType.Silu
                    )
                    sils[j] = sil
                else:
                    ps = pu.tile([P, C], f32, tag="u_ps")
                    for k in range(KD):
                        if e == 0 and step < 3 and k and k % KX == 0:
                            filler(1)
                        nc.tensor.matmul(
                            ps[:],
                            uts[j][:, bass.ts(k, P)],
                            xts[k // kx][:, bass.ts(k % kx, C)],
                            start=(k == 0),
                            stop=(k == KD - 1),
                        )
                    nc.vector.tensor_mul(ht[:, j, :], sils.pop(j)[:], ps[:])
                if e == 0 and step in FILL:
                    filler(FILL[step])

            # ---- down proj: yT[m-chunk, :] = sum_j wd[j,m].T @ hT[j] ----
            dts = dts0 if e == 0 else dts1
            cwb = cwb0 if e == 0 else cwb1
            y = y0 if e == 0 else y1
            ydst = y.rearrange("(m p) c -> p m c", p=P)
            if e < EPC - 1:
                batches = [(0, 4, nc.gpsimd), (4, 4, nc.gpsimd),
                           (8, 4, nc.gpsimd), (12, 4, nc.gpsimd)]
            else:
                # shrink + parallelize the final flush to cut the tail
                batches = [(0, 4, nc.gpsimd), (4, 4, nc.gpsimd),
                           (8, 4, nc.gpsimd), (12, 2, nc.gpsimd),
                           (14, 1, nc.sync), (15, 1, nc.scalar)]
            for m0, nb, yeng in batches:
                ysb = op.tile([P, nb, C], bf16, tag="ysb")
                for mi in range(nb):
                    m = m0 + mi
                    y_ps = py.tile([P, C], f32, tag="y_ps")
                    for j in range(KF):
                        nc.tensor.matmul(
                            y_ps[:],
                            dts[j // (KF // 3)][:, j % (KF // 3), bass.ts(m, P)],
                            ht[:, j, :],
                            start=(j == 0),
                            stop=(j == KF - 1),
                        )
                    nc.vector.tensor_mul(ysb[:, mi, :], y_ps[:], cwb[:])
                yeng.dma_start(ydst[:, m0 : m0 + nb, :], ysb[:])

    _split_waits(nc)
    return nc


_CACHE = {}


def _get_nc(C0, C1):
    if (C0, C1) not in _CACHE:
        _CACHE[(C0, C1)] = build_moe(C0, C1)
    return _CACHE[(C0, C1)]


def _route(x, router_w):
    """Replicates the reference router in f32: softmax over expert scores,
    top-2, renormalize."""
    xf = x.reshape(-1, D).astype(np.float32)
    scores = xf @ router_w.astype(np.float32)
    m = scores.max(axis=-1, keepdims=True)
    ex = np.exp(scores - m)
    probs = ex / ex.sum(axis=-1, keepdims=True)
    idx = np.argsort(-probs, axis=-1, kind="stable")[:, :TOPK]
    wts = np.take_along_axis(probs, idx, axis=-1)
    wts = wts / wts.sum(axis=-1, keepdims=True)
    return idx.astype(np.int32), wts.astype(np.float32)


def _round8(n):
    return max(8, -(-n // 8) * 8)


def kernel(x, router_w, gate_w, up_w, down_w):
    import ml_dtypes

    bf = ml_dtypes.bfloat16

    x = np.asarray(x)
    in_dtype = x.dtype
    xf = x.reshape(-1, D).astype(np.float32)
    idx, wts = _route(x, np.asarray(router_w))

    # token lists per expert
    tok_ids = [None] * E
    tok_wts = [None] * E
    counts = np.zeros(E, dtype=np.int64)
    for e in range(E):
        sel = np.nonzero(idx == e)
        tok_ids[e] = sel[0].astype(np.int64)
        tok_wts[e] = wts[sel[0], sel[1]]
        counts[e] = len(tok_ids[e])

    # load-balanced slots: slot0 = 8 largest experts, slot1 = 8 smallest;
    # core c processes (desc[c], asc[c]).
    order = np.argsort(-counts, kind="stable")
    slot0 = order[:NCORES]
    slot1 = order[NCORES:][::-1]  # ascending counts
    C0 = min(512, _round8(int(counts[slot0].max())))
    C1 = min(512, _round8(int(counts[slot1].max())))

    nc = _get_nc(C0, C1)

    def tile_gateup(w):
        # [E, D, F] -> [E, KF, P, KD*P] with w_t[e,j,p,k*P+f] = w[e,k*P+p,j*P+f]
        w = np.asarray(w).astype(bf)
        w = w.reshape(E, KD, P, KF, P).transpose(0, 3, 2, 1, 4)
        return np.ascontiguousarray(w.reshape(E, KF, P, KD * P))

    g16 = tile_gateup(gate_w)
    u16 = tile_gateup(up_w)
    d16 = np.asarray(down_w).astype(bf)
    xT = np.ascontiguousarray(xf.T)  # [D, B*T] f32

    def gather(e, C):
        xg = np.zeros((P, KD, C), dtype=bf)
        cwv = np.zeros((P, C), dtype=np.float32)
        n = counts[e]
        gath = xT[:, tok_ids[e]]  # [D, n] f32
        xg[:, :, :n] = gath.astype(bf).reshape(KD, P, n).transpose(1, 0, 2)
        cwv[:, :n] = tok_wts[e][None, :]
        return xg.reshape(P, KD * C), cwv

    in_maps = []
    for c in range(NCORES):
        e0, e1 = int(slot0[c]), int(slot1[c])
        xg0, cwv0 = gather(e0, C0)
        xg1, cwv1 = gather(e1, C1)
        pair = [e0, e1]
        in_maps.append(
            {
                "xg0": xg0,
                "xg1": xg1,
                "cw0": cwv0,
                "cw1": cwv1,
                "wg": np.ascontiguousarray(g16[pair]),
                "wu": np.ascontiguousarray(u16[pair]),
                "wd": np.ascontiguousarray(d16[pair]),
            }
        )

    res = run_bass_kernel_spmd(nc, in_maps, list(range(NCORES)))

    out = np.zeros((B * T, D), dtype=np.float32)
    for c in range(NCORES):
        e0, e1 = int(slot0[c]), int(slot1[c])
        for e, name in ((e0, "y0"), (e1, "y1")):
            yv = res.results[c][name]  # [D, C] bf16
            n = counts[e]
            out[tok_ids[e]] += yv[:, :n].astype(np.float32).T
    return out.reshape(B, T, D).astype(in_dtype)


# revision 7
# speedup vs baseline: 1.1187x; 1.0119x over previous
"""MoE layer (B=2,T=1024,D=2048,F=768,E=16,K=2) on 8 NeuronCores.

Expert-parallel with load-balanced slots: slot0 = the 8 largest experts
(capacity C0), slot1 = the 8 smallest (capacity C1 <= C0), one of each per
core. Host computes the router (~0.3% of FLOPs), gathers each expert's
tokens into fixed-capacity transposed buffers, and the device kernel runs
the sparse SwiGLU FFN in bf16 with f32 PSUM accumulation.

Schedule (per core, derived from trace analysis):
- slot0 tokens (4 col-slabs, queued first) + up slabs ride the scalar HWDGE
  ring; gate slabs (gt0 split in two for an early first matmul) + down
  weights + ALL slot1 gate/up/tokens ride the sync ring; cw + slot1 down
  weights + outputs ride gpsimd SWDGE. Only 10 scalar triggers precede the
  silus so the ACT engine is never blocked behind its DMA ring.
- gate/up chunks consumed in zipper order g0 g1 u0 g2 u1 ... u5 to match
  per-ring FIFO arrival; silu runs right after each g chunk.
- PE warmup: memset on gpsimd (earliest engine up) + 8 garbage matmuls,
  plus 256-col filler matmuls at slab boundaries of the first chunks so
  the HAM clock-gate never sees a 3.4us idle window and re-throttles
  (supply dribbles during the DMA ramp; fillers make the stalls cheap).
- balanced slots need only (296+256) matmul columns per core instead of
  2x296: slot0 = the 8 largest experts, slot1 = the 8 smallest.
- the last expert's final two m-chunks go out as two small parallel DMAs
  on sync+scalar to cut the post-compute tail.
"""

import numpy as np
from contextlib import ExitStack

import concourse.bass as bass
import concourse.tile as tile
from concourse import mybir
from concourse.bass_utils import run_bass_kernel_spmd

B, T, D, F, E, TOPK = 2, 1024, 2048, 768, 16, 2
NCORES = 8
EPC = E // NCORES  # expert slots per core
P = 128
KD = D // P  # 16 k-tiles over D
KF = F // P  # 6 f-chunks over F
MD = D // P  # 16 m-chunks over D (down proj, yT layout)


def _split_waits(nc, max_waits=1):
    """walrus on this image rejects >1 sync-wait per instruction
    (setupSyncWait: "Too many sync wait commands"); split extras into
    preceding same-engine NoOps."""
    for f in nc.m.functions:
        for b in f.blocks:
            insts = b.instructions
            idx = 0
            while idx < len(insts):
                inst = insts[idx]
                si = getattr(inst, "sync_info", None)
                if si is not None and si.on_wait and len(si.on_wait) > max_waits:
                    waits = list(si.on_wait)
                    extra, keep = waits[:-max_waits], waits[-max_waits:]
                    pos = idx
                    for j in range(0, len(extra), max_waits):
                        chunk = extra[j : j + max_waits]
                        nop = mybir.InstNoOp(name=f"{inst.name}_ws{j}", ins=[], outs=[])
                        nop.engine = inst.engine
                        nop.sync_info = mybir.SyncInfo(on_wait=chunk, on_update=[])
                        insts.insert(pos, nop)
                        pos += 1
                        idx += 1
                    inst.sync_info = mybir.SyncInfo(
                        on_wait=keep, on_update=list(si.on_update)
                    )
                idx += 1


def build_moe(C0, C1):
    """Per-core kernel: slot0 capacity C0, slot1 capacity C1 (both %8==0)."""
    assert C0 % 8 == 0 and C1 % 8 == 0 and C1 <= C0 <= 512
    caps = (C0, C1)
    bf16 = mybir.dt.bfloat16
    f32 = mybir.dt.float32
    H = P // 2  # partition-stripe boundary

    nc = bass.Bass("TRN2", target_bir_lowering=False, debug=False, num_devices=NCORES)
    # host pre-tiled layouts (>=2KB contiguous per partition per DMA):
    #   xg[p, k*C + c] = x_gathered[k*128+p, c]
    #   wg/wu[e, j, p, k*128+f] = w[e, k*128+p, j*128+f]   (slab per f-chunk j)
    xg0 = nc.declare_dram_parameter("xg0", [P, KD * C0], bf16, isOutput=False)
    xg1 = nc.declare_dram_parameter("xg1", [P, KD * C1], bf16, isOutput=False)
    cw0 = nc.declare_dram_parameter("cw0", [P, C0], f32, isOutput=False)
    cw1 = nc.declare_dram_parameter("cw1", [P, C1], f32, isOutput=False)
    wg = nc.declare_dram_parameter("wg", [EPC, KF, P, KD * P], bf16, isOutput=False)
    wu = nc.declare_dram_parameter("wu", [EPC, KF, P, KD * P], bf16, isOutput=False)
    wd = nc.declare_dram_parameter("wd", [EPC, F, D], bf16, isOutput=False)
    y0 = nc.declare_dram_parameter("y0", [D, C0], bf16, isOutput=True)
    y1 = nc.declare_dram_parameter("y1", [D, C1], bf16, isOutput=True)

    with tile.TileContext(nc) as tc, ExitStack() as ctx:
        xp = ctx.enter_context(tc.tile_pool(name="xp", bufs=1))
        wp = ctx.enter_context(tc.tile_pool(name="wp", bufs=1))
        wdp = ctx.enter_context(tc.tile_pool(name="wdp", bufs=1))
        hp = ctx.enter_context(tc.tile_pool(name="hp", bufs=1))
        sp = ctx.enter_context(tc.tile_pool(name="sp", bufs=3))
        cp = ctx.enter_context(tc.tile_pool(name="cp", bufs=1))
        op = ctx.enter_context(tc.tile_pool(name="op", bufs=4))
        pg = ctx.enter_context(tc.tile_pool(name="pg", bufs=3, space="PSUM"))
        pu = ctx.enter_context(tc.tile_pool(name="pu", bufs=2, space="PSUM"))
        py = ctx.enter_context(tc.tile_pool(name="py", bufs=3, space="PSUM"))

        # PE warmup: garbage matmuls with no data deps run during the DMA
        # ramp so HAM un-throttles (1.2->2.4GHz) before real work. memset on
        # gpsimd -- the earliest engine to come up after the preamble.
        wsb = cp.tile([P, 512], bf16, tag="warm_sb")
        nc.gpsimd.memset(wsb[:], 0)
        for _ in range(12):
            wps = py.tile([P, 512], f32, tag="y_ps")
            nc.tensor.matmul(wps[:], wsb[:, :P], wsb[:], start=True, stop=True)

        def filler(n, cols=512):
            for _ in range(n):
                wps = py.tile([P, 512], f32, tag="y_ps")
                nc.tensor.matmul(
                    wps[:, :cols], wsb[:, :P], wsb[:, :cols], start=True, stop=True
                )

        # ---- slot0 loads: e0-critical bytes (tokens + gate/up slabs)
        # dealt round-robin across all 3 DMA rings in exact demand order, so
        # aggregate early bandwidth goes 100% to what the PE needs next.
        XC = 4  # token column chunks (KD/XC k-tiles each)
        KX = KD // XC
        xts0 = []
        for h in range(XC):
            xt = xp.tile([P, KX * C0], bf16, tag=f"xt0_{h}")
            xts0.append(xt)
        gts0, uts0 = [], []
        for j in range(KF):
            gt = wp.tile([P, KD * P], bf16, tag=f"gt0_{j}")
            gts0.append(gt)
            ut = wp.tile([P, KD * P], bf16, tag=f"ut0_{j}")
            uts0.append(ut)

        cwb0 = cp.tile([P, C0], f32, tag="cwb0")
        nc.gpsimd.dma_start(cwb0[:], cw0[:])

        items = [
            (xts0[0][:], xg0[:, 0 : KX * C0]),
            (gts0[0][:, 0 : 8 * P], wg[0, 0][:, 0 : 8 * P]),
            (gts0[0][:, 8 * P :], wg[0, 0][:, 8 * P :]),
            (xts0[1][:], xg0[:, KX * C0 : 2 * KX * C0]),
            (xts0[2][:], xg0[:, 2 * KX * C0 : 3 * KX * C0]),
            (xts0[3][:], xg0[:, 3 * KX * C0 :]),
            (gts0[1][:], wg[0, 1]),
            (uts0[0][:], wu[0, 0]),
            (gts0[2][:], wg[0, 2]),
            (uts0[1][:], wu[0, 1]),
            (gts0[3][:], wg[0, 3]),
            (uts0[2][:], wu[0, 2]),
            (gts0[4][:], wg[0, 4]),
            (uts0[3][:], wu[0, 3]),
            (gts0[5][:], wg[0, 5]),
            (uts0[4][:], wu[0, 4]),
            (uts0[5][:], wu[0, 5]),
        ]
        engs = [nc.scalar, nc.sync, nc.gpsimd]
        for i, (dst, src) in enumerate(items):
            engs[i % 3].dma_start(dst, src)

        # slot0 down weights + all slot1 gate/up/tokens behind them on sync
        dts0 = []
        wdr0 = wd[0].rearrange("(k p) d -> p k d", p=P)
        for h in range(3):
            dt = wdp.tile([P, KF // 3, D], bf16, tag=f"dt0_{h}")
            nc.sync.dma_start(dt[:], wdr0[:, bass.ts(h, KF // 3), :])
            dts0.append(dt)
        xt1 = xp.tile([P, KD * C1], bf16, tag="xt1")
        nc.sync.dma_start(xt1[:], xg1[:])
        gts1, uts1 = [], []
        for j in range(KF):
            gt = wp.tile([P, KD * P], bf16, tag=f"gt1_{j}")
            gts1.append(gt)
            ut = wp.tile([P, KD * P], bf16, tag=f"ut1_{j}")
            uts1.append(ut)
        nc.sync.dma_start(gts1[0][:], wg[1, 0])
        nc.sync.dma_start(gts1[1][:], wg[1, 1])
        nc.sync.dma_start(uts1[0][:], wu[1, 0])
        nc.sync.dma_start(gts1[2][:], wg[1, 2])
        nc.sync.dma_start(uts1[1][:], wu[1, 1])
        nc.sync.dma_start(gts1[3][:], wg[1, 3])
        nc.sync.dma_start(uts1[2][:], wu[1, 2])
        nc.sync.dma_start(gts1[4][:], wg[1, 4])
        nc.sync.dma_start(uts1[3][:], wu[1, 3])
        nc.sync.dma_start(gts1[5][:], wg[1, 5])
        nc.sync.dma_start(uts1[4][:], wu[1, 4])
        nc.sync.dma_start(uts1[5][:], wu[1, 5])

        # slot1 down weights on scalar (drained long before silu pressure
        # ends; keeps gpsimd free for outputs and sync for slot1 gate/up)
        dts1 = []
        wdr1 = wd[1].rearrange("(k p) d -> p k d", p=P)
        for h in range(3):
            dt = wdp.tile([P, KF // 3, D], bf16, tag=f"dt1_{h}")
            nc.scalar.dma_start(dt[:], wdr1[:, bass.ts(h, KF // 3), :])
            dts1.append(dt)
        cwb1 = cp.tile([P, C1], f32, tag="cwb1")
        nc.gpsimd.dma_start(cwb1[:], cw1[:])

        ZIP = [("g", 0), ("g", 1), ("u", 0), ("g", 2), ("u", 1), ("g", 3),
               ("u", 2), ("g", 4), ("u", 3), ("g", 5), ("u", 4), ("u", 5)]
        # fillers after the i-th zipper step of slot0 (supply-bound ramp)
        FILL = {0: 4, 1: 3, 2: 3, 3: 2, 4: 1, 5: 1}

        hts = []
        for e in range(EPC):
            C = caps[e]
            xts = xts0 if e == 0 else [xt1]
            kx = KX if e == 0 else KD
            gts = gts0 if e == 0 else gts1
            uts = uts0 if e == 0 else uts1

            # ---- gate/up + SwiGLU -> hT [F, C] bf16, zipper order ----
            ht = hp.tile([P, KF, C], bf16, tag=f"ht{e}")
            hts.append(ht)
            sils = {}
            for step, (kind, j) in enumerate(ZIP):
                if kind == "g":
                    ps = pg.tile([P, C], f32, tag="g_ps")
                    for k in range(KD):
                        if e == 0 and step == 0 and k and k % KX == 0:
                            filler(4, cols=512)
                        nc.tensor.matmul(
                            ps[:],
                            gts[j][:, bass.ts(k, P)],
                            xts[k // kx][:, bass.ts(k % kx, C)],
                            start=(k == 0),
                            stop=(k == KD - 1),
                        )
                    sil = sp.tile([P, C], f32, tag="sil")
                    nc.scalar.activation(
                        sil[:], ps[:], mybir.ActivationFunctionType.Silu
                    )
                    sils[j] = sil
                else:
                    ps = pu.tile([P, C], f32, tag="u_ps")
                    for k in range(KD):
                        nc.tensor.matmul(
                            ps[:],
                            uts[j][:, bass.ts(k, P)],
                            xts[k // kx][:, bass.ts(k % kx, C)],
                            start=(k == 0),
                            stop=(k == KD - 1),
                        )
                    nc.vector.tensor_mul(ht[:, j, :], sils.pop(j)[:], ps[:])
                if e == 0 and step in FILL:
                    filler(FILL[step])

            # ---- down proj: yT[m-chunk, :] = sum_j wd[j,m].T @ hT[j] ----
            dts = dts0 if e == 0 else dts1
            cwb = cwb0 if e == 0 else cwb1
            y = y0 if e == 0 else y1
            ydst = y.rearrange("(m p) c -> p m c", p=P)
            if e < EPC - 1:
                batches = [(0, 4, nc.gpsimd), (4, 4, nc.gpsimd),
                           (8, 4, nc.gpsimd), (12, 4, nc.gpsimd)]
            else:
                # shrink + parallelize the final flush to cut the tail
                batches = [(0, 4, nc.gpsimd), (4, 4, nc.gpsimd),
                           (8, 4, nc.gpsimd), (12, 2, nc.gpsimd),
                           (14, 1, nc.sync), (15, 1, nc.scalar)]
            for m0, nb, yeng in batches:
                ysb = op.tile([P, nb, C], bf16, tag="ysb")
                for mi in range(nb):
                    m = m0 + mi
                    y_ps = py.tile([P, C], f32, tag="y_ps")
                    for j in range(KF):
                        nc.tensor.matmul(
                            y_ps[:],
                            dts[j // (KF // 3)][:, j % (KF // 3), bass.ts(m, P)],
                            ht[:, j, :],
                            start=(j == 0),
                            stop=(j == KF - 1),
                        )
                    nc.vector.tensor_mul(ysb[:, mi, :], y_ps[:], cwb[:])
                yeng.dma_start(ydst[:, m0 : m0 + nb, :], ysb[:])

    _split_waits(nc)
    return nc


_CACHE = {}


def _get_nc(C0, C1):
    if (C0, C1) not in _CACHE:
        _CACHE[(C0, C1)] = build_moe(C0, C1)
    return _CACHE[(C0, C1)]


def _route(x, router_w):
    """Replicates the reference router in f32: softmax over expert scores,
    top-2, renormalize."""
    xf = x.reshape(-1, D).astype(np.float32)
    scores = xf @ router_w.astype(np.float32)
    m = scores.max(axis=-1, keepdims=True)
    ex = np.exp(scores - m)
    probs = ex / ex.sum(axis=-1, keepdims=True)
    idx = np.argsort(-probs, axis=-1, kind="stable")[:, :TOPK]
    wts = np.take_along_axis(probs, idx, axis=-1)
    wts = wts / wts.sum(axis=-1, keepdims=True)
    return idx.astype(np.int32), wts.astype(np.float32)


def _round8(n):
    return max(8, -(-n // 8) * 8)


def kernel(x, router_w, gate_w, up_w, down_w):
    import ml_dtypes

    bf = ml_dtypes.bfloat16

    x = np.asarray(x)
    in_dtype = x.dtype
    xf = x.reshape(-1, D).astype(np.float32)
    idx, wts = _route(x, np.asarray(router_w))

    # token lists per expert
    tok_ids = [None] * E
    tok_wts = [None] * E
    counts = np.zeros(E, dtype=np.int64)
    for e in range(E):
        sel = np.nonzero(idx == e)
        tok_ids[e] = sel[0].astype(np.int64)
        tok_wts[e] = wts[sel[0], sel[1]]
        counts[e] = len(tok_ids[e])

    # load-balanced slots: slot0 = 8 largest experts, slot1 = 8 smallest;
    # core c processes (desc[c], asc[c]).
    order = np.argsort(-counts, kind="stable")
    slot0 = order[:NCORES]
    slot1 = order[NCORES:][::-1]  # ascending counts
    C0 = min(512, _round8(int(counts[slot0].max())))
    C1 = min(512, _round8(int(counts[slot1].max())))

    nc = _get_nc(C0, C1)

    def tile_gateup(w):
        # [E, D, F] -> [E, KF, P, KD*P] with w_t[e,j,p,k*P+f] = w[e,k*P+p,j*P+f]
        w = np.asarray(w).astype(bf)
        w = w.reshape(E, KD, P, KF, P).transpose(0, 3, 2, 1, 4)
        return np.ascontiguousarray(w.reshape(E, KF, P, KD * P))

    g16 = tile_gateup(gate_w)
    u16 = tile_gateup(up_w)
    d16 = np.asarray(down_w).astype(bf)
    xT = np.ascontiguousarray(xf.T)  # [D, B*T] f32

    def gather(e, C):
        xg = np.zeros((P, KD, C), dtype=bf)
        cwv = np.zeros((P, C), dtype=np.float32)
        n = counts[e]
        gath = xT[:, tok_ids[e]]  # [D, n] f32
        xg[:, :, :n] = gath.astype(bf).reshape(KD, P, n).transpose(1, 0, 2)
        cwv[:, :n] = tok_wts[e][None, :]
        return xg.reshape(P, KD * C), cwv

    in_maps = []
    for c in range(NCORES):
        e0, e1 = int(slot0[c]), int(slot1[c])
        xg0, cwv0 = gather(e0, C0)
        xg1, cwv1 = gather(e1, C1)
        pair = [e0, e1]
        in_maps.append(
            {
                "xg0": xg0,
                "xg1": xg1,
                "cw0": cwv0,
                "cw1": cwv1,
                "wg": np.ascontiguousarray(g16[pair]),
                "wu": np.ascontiguousarray(u16[pair]),
                "wd": np.ascontiguousarray(d16[pair]),
            }
        )

    res = run_bass_kernel_spmd(nc, in_maps, list(range(NCORES)))

    out = np.zeros((B * T, D), dtype=np.float32)
    for c in range(NCORES):
        e0, e1 = int(slot0[c]), int(slot1[c])
        for e, name in ((e0, "y0"), (e1, "y1")):
            yv = res.results[c][name]  # [D, C] bf16
            n = counts[e]
            out[tok_ids[e]] += yv[:, :n].astype(np.float32).T
    return out.reshape(B, T, D).astype(in_dtype)


# revision 9
# speedup vs baseline: 1.1216x; 1.0026x over previous
"""MoE layer (B=2,T=1024,D=2048,F=768,E=16,K=2) on 8 NeuronCores.

Expert-parallel with load-balanced slots: slot0 = the 8 largest experts
(capacity C0), slot1 = the 8 smallest (capacity C1 <= C0), one of each per
core. Host computes the router (~0.3% of FLOPs), gathers each expert's
tokens into fixed-capacity transposed buffers, and the device kernel runs
the sparse SwiGLU FFN in bf16 with f32 PSUM accumulation.

Schedule (per core, derived from trace analysis):
- slot0 tokens (4 col-slabs, queued first) + up slabs ride the scalar HWDGE
  ring; gate slabs (gt0 split in two for an early first matmul) + down
  weights + ALL slot1 gate/up/tokens ride the sync ring; cw + slot1 down
  weights + outputs ride gpsimd SWDGE. Only 10 scalar triggers precede the
  silus so the ACT engine is never blocked behind its DMA ring.
- gate/up chunks consumed in zipper order g0 g1 u0 g2 u1 ... u5 to match
  per-ring FIFO arrival; silu runs right after each g chunk.
- PE warmup: memset on gpsimd (earliest engine up) + 8 garbage matmuls,
  plus 256-col filler matmuls at slab boundaries of the first chunks so
  the HAM clock-gate never sees a 3.4us idle window and re-throttles
  (supply dribbles during the DMA ramp; fillers make the stalls cheap).
- balanced slots need only (296+256) matmul columns per core instead of
  2x296: slot0 = the 8 largest experts, slot1 = the 8 smallest.
- the last expert's final two m-chunks go out as two small parallel DMAs
  on sync+scalar to cut the post-compute tail.
"""

import numpy as np
from contextlib import ExitStack

import concourse.bass as bass
import concourse.tile as tile
from concourse import mybir
from concourse.bass_utils import run_bass_kernel_spmd

B, T, D, F, E, TOPK = 2, 1024, 2048, 768, 16, 2
NCORES = 8
EPC = E // NCORES  # expert slots per core
P = 128
KD = D // P  # 16 k-tiles over D
KF = F // P  # 6 f-chunks over F
MD = D // P  # 16 m-chunks over D (down proj, yT layout)


def _split_waits(nc, max_waits=1):
    """walrus on this image rejects >1 sync-wait per instruction
    (setupSyncWait: "Too many sync wait commands"); split extras into
    preceding same-engine NoOps."""
    for f in nc.m.functions:
        for b in f.blocks:
            insts = b.instructions
            idx = 0
            while idx < len(insts):
                inst = insts[idx]
                si = getattr(inst, "sync_info", None)
                if si is not None and si.on_wait and len(si.on_wait) > max_waits:
                    waits = list(si.on_wait)
                    extra, keep = waits[:-max_waits], waits[-max_waits:]
                    pos = idx
                    for j in range(0, len(extra), max_waits):
                        chunk = extra[j : j + max_waits]
                        nop = mybir.InstNoOp(name=f"{inst.name}_ws{j}", ins=[], outs=[])
                        nop.engine = inst.engine
                        nop.sync_info = mybir.SyncInfo(on_wait=chunk, on_update=[])
                        insts.insert(pos, nop)
                        pos += 1
                        idx += 1
                    inst.sync_info = mybir.SyncInfo(
                        on_wait=keep, on_update=list(si.on_update)
                    )
                idx += 1


def build_moe(C0, C1):
    """Per-core kernel: slot0 capacity C0, slot1 capacity C1 (both %8==0)."""
    assert C0 % 8 == 0 and C1 % 8 == 0 and C1 <= C0 <= 512
    caps = (C0, C1)
    bf16 = mybir.dt.bfloat16
    f32 = mybir.dt.float32
    H = P // 2  # partition-stripe boundary

    nc = bass.Bass("TRN2", target_bir_lowering=False, debug=False, num_devices=NCORES)
    # host pre-tiled layouts (>=2KB contiguous per partition per DMA):
    #   xg[p, k*C + c] = x_gathered[k*128+p, c]
    #   wg/wu[e, j, p, k*128+f] = w[e, k*128+p, j*128+f]   (slab per f-chunk j)
    xg0 = nc.declare_dram_parameter("xg0", [P, KD * C0], bf16, isOutput=False)
    xg1 = nc.declare_dram_parameter("xg1", [P, KD * C1], bf16, isOutput=False)
    cw0 = nc.declare_dram_parameter("cw0", [P, C0], f32, isOutput=False)
    cw1 = nc.declare_dram_parameter("cw1", [P, C1], f32, isOutput=False)
    wg = nc.declare_dram_parameter("wg", [EPC, KF, P, KD * P], bf16, isOutput=False)
    wu = nc.declare_dram_parameter("wu", [EPC, KF, P, KD * P], bf16, isOutput=False)
    wd = nc.declare_dram_parameter("wd", [EPC, F, D], bf16, isOutput=False)
    y0 = nc.declare_dram_parameter("y0", [D, C0], bf16, isOutput=True)
    y1 = nc.declare_dram_parameter("y1", [D, C1], bf16, isOutput=True)

    with tile.TileContext(nc) as tc, ExitStack() as ctx:
        xp = ctx.enter_context(tc.tile_pool(name="xp", bufs=1))
        wp = ctx.enter_context(tc.tile_pool(name="wp", bufs=1))
        wdp = ctx.enter_context(tc.tile_pool(name="wdp", bufs=1))
        hp = ctx.enter_context(tc.tile_pool(name="hp", bufs=1))
        sp = ctx.enter_context(tc.tile_pool(name="sp", bufs=3))
        cp = ctx.enter_context(tc.tile_pool(name="cp", bufs=1))
        op = ctx.enter_context(tc.tile_pool(name="op", bufs=4))
        pg = ctx.enter_context(tc.tile_pool(name="pg", bufs=3, space="PSUM"))
        pu = ctx.enter_context(tc.tile_pool(name="pu", bufs=2, space="PSUM"))
        py = ctx.enter_context(tc.tile_pool(name="py", bufs=3, space="PSUM"))

        # PE warmup: garbage matmuls with no data deps run during the DMA
        # ramp so HAM un-throttles (1.2->2.4GHz) before real work. memset on
        # gpsimd -- the earliest engine to come up after the preamble.
        wsb = cp.tile([P, 512], bf16, tag="warm_sb")
        nc.gpsimd.memset(wsb[:], 0)
        for _ in range(12):
            wps = py.tile([P, 512], f32, tag="y_ps")
            nc.tensor.matmul(wps[:], wsb[:, :P], wsb[:], start=True, stop=True)

        def filler(n, cols=512):
            for _ in range(n):
                wps = py.tile([P, 512], f32, tag="y_ps")
                nc.tensor.matmul(
                    wps[:, :cols], wsb[:, :P], wsb[:, :cols], start=True, stop=True
                )

        # ---- loads: one global demand-ordered stream, dealt round-robin
        # across the 3 DMA rings so aggregate early bandwidth always goes to
        # the bytes the PE needs next. Per-ring FIFO preserves the order.
        XC = 4  # token column chunks (KD/XC k-tiles each)
        KX = KD // XC
        xts0 = []
        for h in range(XC):
            xt = xp.tile([P, KX * C0], bf16, tag=f"xt0_{h}")
            xts0.append(xt)
        gts0, uts0, gts1, uts1 = [], [], [], []
        for j in range(KF):
            gt = wp.tile([P, KD * P], bf16, tag=f"gt0_{j}")
            gts0.append(gt)
            ut = wp.tile([P, KD * P], bf16, tag=f"ut0_{j}")
            uts0.append(ut)
            gt1 = wp.tile([P, KD * P], bf16, tag=f"gt1_{j}")
            gts1.append(gt1)
            ut1 = wp.tile([P, KD * P], bf16, tag=f"ut1_{j}")
            uts1.append(ut1)
        xt1 = xp.tile([P, KD * C1], bf16, tag="xt1")
        dts0, dts1 = [], []
        wdr0 = wd[0].rearrange("(k p) d -> p k d", p=P)
        wdr1 = wd[1].rearrange("(k p) d -> p k d", p=P)
        for h in range(3):
            dt = wdp.tile([P, KF // 3, D], bf16, tag=f"dt0_{h}")
            dts0.append(dt)
            dt1 = wdp.tile([P, KF // 3, D], bf16, tag=f"dt1_{h}")
            dts1.append(dt1)
        cwb0 = cp.tile([P, C0], f32, tag="cwb0")
        nc.gpsimd.dma_start(cwb0[:], cw0[:])

        items = [
            (xts0[0][:], xg0[:, 0 : KX * C0]),
            (gts0[0][:, 0 : 8 * P], wg[0, 0][:, 0 : 8 * P]),
            (gts0[0][:, 8 * P :], wg[0, 0][:, 8 * P :]),
            (xts0[1][:], xg0[:, KX * C0 : 2 * KX * C0]),
            (xts0[2][:], xg0[:, 2 * KX * C0 : 3 * KX * C0]),
            (xts0[3][:], xg0[:, 3 * KX * C0 :]),
            (gts0[1][:], wg[0, 1]),
            (uts0[0][:], wu[0, 0]),
            (gts0[2][:], wg[0, 2]),
            (uts0[1][:], wu[0, 1]),
            (gts0[3][:], wg[0, 3]),
            (uts0[2][:], wu[0, 2]),
            (gts0[4][:], wg[0, 4]),
            (uts0[3][:], wu[0, 3]),
            (gts0[5][:], wg[0, 5]),
            (uts0[4][:], wu[0, 4]),
            (uts0[5][:], wu[0, 5]),
            # e1's first two gate chunks cover the dt0 arrival window
            (xt1[:], xg1[:]),
            (gts1[0][:], wg[1, 0]),
            (gts1[1][:], wg[1, 1]),
            (dts0[0][:], wdr0[:, 0 : KF // 3, :]),
            (dts0[1][:], wdr0[:, KF // 3 : 2 * (KF // 3), :]),
            (dts0[2][:], wdr0[:, 2 * (KF // 3) :, :]),
        ]
        engs = [nc.scalar, nc.sync, nc.gpsimd]
        for i, (dst, src) in enumerate(items):
            engs[i % 3].dma_start(dst, src)

        # slot1 remainder on sync, zipper order
        nc.sync.dma_start(uts1[0][:], wu[1, 0])
        nc.sync.dma_start(gts1[2][:], wg[1, 2])
        nc.sync.dma_start(uts1[1][:], wu[1, 1])
        nc.sync.dma_start(gts1[3][:], wg[1, 3])
        nc.sync.dma_start(uts1[2][:], wu[1, 2])
        nc.sync.dma_start(gts1[4][:], wg[1, 4])
        nc.sync.dma_start(uts1[3][:], wu[1, 3])
        nc.sync.dma_start(gts1[5][:], wg[1, 5])
        nc.sync.dma_start(uts1[4][:], wu[1, 4])
        nc.sync.dma_start(uts1[5][:], wu[1, 5])
        # slot1 down weights split across ring tails (needed ~70us)
        nc.scalar.dma_start(dts1[0][:], wdr1[:, 0 : KF // 3, :])
        nc.gpsimd.dma_start(dts1[2][:], wdr1[:, 2 * (KF // 3) :, :])
        nc.sync.dma_start(dts1[1][:], wdr1[:, KF // 3 : 2 * (KF // 3), :])
        cwb1 = cp.tile([P, C1], f32, tag="cwb1")
        nc.gpsimd.dma_start(cwb1[:], cw1[:])

        # ---- compute: chunk emitters ----
        ZIP = [("g", 0), ("g", 1), ("u", 0), ("g", 2), ("u", 1), ("g", 3),
               ("u", 2), ("g", 4), ("u", 3), ("g", 5), ("u", 4), ("u", 5)]
        FILL = {0: 4, 1: 3, 2: 3, 3: 2, 4: 1, 5: 1}
        ht0 = hp.tile([P, KF, C0], bf16, tag="ht0")
        ht1 = hp.tile([P, KF, C1], bf16, tag="ht1")
        hts = [ht0, ht1]
        sils = [{}, {}]

        def gu_chunk(e, kind, j, fill_k=False):
            C = caps[e]
            xts = xts0 if e == 0 else [xt1]
            kx = KX if e == 0 else KD
            if kind == "g":
                ps = pg.tile([P, C], f32, tag="g_ps")
                wts_ = (gts0 if e == 0 else gts1)[j]
            else:
                ps = pu.tile([P, C], f32, tag="u_ps")
                wts_ = (uts0 if e == 0 else uts1)[j]
            for k in range(KD):
                if fill_k and k and k % KX == 0:
                    filler(4)
                nc.tensor.matmul(
                    ps[:],
                    wts_[:, bass.ts(k, P)],
                    xts[k // kx][:, bass.ts(k % kx, C)],
                    start=(k == 0),
                    stop=(k == KD - 1),
                )
            if kind == "g":
                sil = sp.tile([P, C], f32, tag="sil")
                nc.scalar.activation(sil[:], ps[:], mybir.ActivationFunctionType.Silu)
                sils[e][j] = sil
            else:
                nc.vector.tensor_mul(hts[e][:, j, :], sils[e].pop(j)[:], ps[:])

        def down(e):
            C = caps[e]
            dts = dts0 if e == 0 else dts1
            cwb = cwb0 if e == 0 else cwb1
            y = y0 if e == 0 else y1
            ydst = y.rearrange("(m p) c -> p m c", p=P)
            if e < EPC - 1:
                batches = [(0, 4, nc.gpsimd), (4, 4, nc.gpsimd),
                           (8, 4, nc.gpsimd), (12, 4, nc.gpsimd)]
            else:
                batches = [(0, 4, nc.gpsimd), (4, 4, nc.gpsimd),
                           (8, 4, nc.gpsimd), (12, 2, nc.gpsimd),
                           (14, 1, nc.sync), (15, 1, nc.scalar)]
            for m0, nb, yeng in batches:
                ysb = op.tile([P, nb, C], bf16, tag="ysb")
                for mi in range(nb):
                    m = m0 + mi
                    y_ps = py.tile([P, C], f32, tag="y_ps")
                    for j in range(KF):
                        nc.tensor.matmul(
                            y_ps[:],
                            dts[j // (KF // 3)][:, j % (KF // 3), bass.ts(m, P)],
                            hts[e][:, j, :],
                            start=(j == 0),
                            stop=(j == KF - 1),
                        )
                    nc.vector.tensor_mul(ysb[:, mi, :], y_ps[:], cwb[:])
                yeng.dma_start(ydst[:, m0 : m0 + nb, :], ysb[:])

        # ---- PE schedule: e0 zipper, e1's first two gate chunks (cover the
        # dt0 arrival window), e0 down, e1 zipper rest, e1 down ----
        for step, (kind, j) in enumerate(ZIP):
            gu_chunk(0, kind, j, fill_k=(step == 0))
            if step in FILL:
                filler(FILL[step])
        gu_chunk(1, "g", 0)
        gu_chunk(1, "g", 1)
        down(0)
        for kind, j in ZIP[2:]:
            gu_chunk(1, kind, j)
        down(1)

    _split_waits(nc)
    return nc


_CACHE = {}


def _get_nc(C0, C1):
    if (C0, C1) not in _CACHE:
        _CACHE[(C0, C1)] = build_moe(C0, C1)
    return _CACHE[(C0, C1)]


def _route(x, router_w):
    """Replicates the reference router in f32: softmax over expert scores,
    top-2, renormalize."""
    xf = x.reshape(-1, D).astype(np.float32)
    scores = xf @ router_w.astype(np.float32)
    m = scores.max(axis=-1, keepdims=True)
    ex = np.exp(scores - m)
    probs = ex / ex.sum(axis=-1, keepdims=True)
    idx = np.argsort(-probs, axis=-1, kind="stable")[:, :TOPK]
    wts = np.take_along_axis(probs, idx, axis=-1)
    wts = wts / wts.sum(axis=-1, keepdims=True)
    return idx.astype(np.int32), wts.astype(np.float32)


def _round8(n):
    return max(8, -(-n // 8) * 8)


def kernel(x, router_w, gate_w, up_w, down_w):
    import ml_dtypes

    bf = ml_dtypes.bfloat16

    x = np.asarray(x)
    in_dtype = x.dtype
    xf = x.reshape(-1, D).astype(np.float32)
    idx, wts = _route(x, np.asarray(router_w))

    # token lists per expert
    tok_ids = [None] * E
    tok_wts = [None] * E
    counts = np.zeros(E, dtype=np.int64)
    for e in range(E):
        sel = np.nonzero(idx == e)
        tok_ids[e] = sel[0].astype(np.int64)
        tok_wts[e] = wts[sel[0], sel[1]]
        counts[e] = len(tok_ids[e])

    # load-balanced slots: slot0 = 8 largest experts, slot1 = 8 smallest;
    # core c processes (desc[c], asc[c]).
    order = np.argsort(-counts, kind="stable")
    slot0 = order[:NCORES]
    slot1 = order[NCORES:][::-1]  # ascending counts
    C0 = min(512, _round8(int(counts[slot0].max())))
    C1 = min(512, _round8(int(counts[slot1].max())))

    nc = _get_nc(C0, C1)

    def tile_gateup(w):
        # [E, D, F] -> [E, KF, P, KD*P] with w_t[e,j,p,k*P+f] = w[e,k*P+p,j*P+f]
        w = np.asarray(w).astype(bf)
        w = w.reshape(E, KD, P, KF, P).transpose(0, 3, 2, 1, 4)
        return np.ascontiguousarray(w.reshape(E, KF, P, KD * P))

    g16 = tile_gateup(gate_w)
    u16 = tile_gateup(up_w)
    d16 = np.asarray(down_w).astype(bf)
    xT = np.ascontiguousarray(xf.T)  # [D, B*T] f32

    def gather(e, C):
        xg = np.zeros((P, KD, C), dtype=bf)
        cwv = np.zeros((P, C), dtype=np.float32)
        n = counts[e]
        gath = xT[:, tok_ids[e]]  # [D, n] f32
        xg[:, :, :n] = gath.astype(bf).reshape(KD, P, n).transpose(1, 0, 2)
        cwv[:, :n] = tok_wts[e][None, :]
        return xg.reshape(P, KD * C), cwv

    in_maps = []
    for c in range(NCORES):
        e0, e1 = int(slot0[c]), int(slot1[c])
        xg0, cwv0 = gather(e0, C0)
        xg1, cwv1 = gather(e1, C1)
        pair = [e0, e1]
        in_maps.append(
            {
                "xg0": xg0,
                "xg1": xg1,
                "cw0": cwv0,
                "cw1": cwv1,
                "wg": np.ascontiguousarray(g16[pair]),
                "wu": np.ascontiguousarray(u16[pair]),
                "wd": np.ascontiguousarray(d16[pair]),
            }
        )

    res = run_bass_kernel_spmd(nc, in_maps, list(range(NCORES)))

    out = np.zeros((B * T, D), dtype=np.float32)
    for c in range(NCORES):
        e0, e1 = int(slot0[c]), int(slot1[c])
        for e, name in ((e0, "y0"), (e1, "y1")):
            yv = res.results[c][name]  # [D, C] bf16
            n = counts[e]
            out[tok_ids[e]] += yv[:, :n].astype(np.float32).T
    return out.reshape(B, T, D).astype(in_dtype)
